# revision 6
# baseline (speedup 1.0000x reference)
"""BatchHardTripletLoss on 8 TRN2 NeuronCores (Bass/Tile).

Wall-clock through the axon tunnel is transfer-dominated (~40 MB/s), so the
design minimizes bytes on the wire:

  - Each core ships ONE bf16 tensor pack[144, 1024] holding its 1/8 of the
    batch: rows 0..127 = e^T, rows 128..137 = one-hot(label), rows 138..139 =
    -||e||^2 split into two bf16 limbs, rows 140..143 = 0.  288 KB/core vs
    11.1 MB/core for a replicated layout.
  - On chip, an AllGather (DRAM bounce buffers) rebuilds the full candidate
    matrices [128, 8192] + [16, 8192] on every core.
  - One matmul pair per (anchor-tile, 512-wide j-chunk) computes
        val[a, j] = 2 e_a.e_j - sq_j - BIG * same(a, j)
    by augmenting the contraction: main matmul contracts the 128 embedding
    rows (lhs = 2*e_a), aux matmul contracts the 16 aux rows
    (lhs = [-BIG*onehot_a; 1; 1; 0*4], rhs = [onehot_j; -sq_hi; -sq_lo; 0*4])
    accumulating into the same PSUM bank.  Since masked same-class entries
    (~ -32.7k) always sit far below real different-class values (>= -600),
        reduce_max_j val  -> hardest-negative partial  max_j(2e.e - sq_j)
        reduce_min_j val  -> hardest-positive partial  min_j(2e.e - sq_j) - BIG
    i.e. ONE value matrix serves both reductions.
  - Output is just [128, 16] f32 per core (hn/hp partials per anchor tile);
    the host finishes with sqrt/relu/mean in float64:
        hn = sqrt(sq_a - hn_m),  hp = sqrt(sq_a - (hp_m + BIG)),
        loss = mean(relu(hp - hn + 1)).
"""

import numpy as np
import ml_dtypes

import concourse.bass as bass
import concourse.bacc as bacc
import concourse.tile as tile
from concourse import mybir
from concourse.bass_utils import run_bass_kernel_spmd

B, D, NCLASS = 8192, 128, 10
NCORES = 8
S = B // NCORES            # 1024 rows per core
NAUX = 12                  # aux rows: 10 one-hot + 2 sq limbs
PACKP = D + NAUX           # 140
NAT = S // 128             # 8 anchor tiles per core
JC = 512                   # j-chunk width (one PSUM bank)
NJC = B // JC              # 16 chunks
BIG = 32768.0              # exact in bf16 (2^15)
MARGIN = 1.0
F32 = mybir.dt.float32
BF16 = mybir.dt.bfloat16
ALU = mybir.AluOpType
BFNP = np.dtype(ml_dtypes.bfloat16)

_NC_CACHE = None


def build_nc():
    nc = bacc.Bacc(num_devices=NCORES)
    pack_d = nc.dram_tensor("pack", [PACKP, S], BF16, kind="ExternalInput")
    res_d = nc.dram_tensor("res", [128, 2 * NAT], F32, kind="ExternalOutput")

    with tile.TileContext(nc) as tc:
        with (
            tc.tile_pool(name="sb", bufs=1) as sb,
            tc.tile_pool(name="dram", bufs=1, space="DRAM") as dram,
            tc.tile_pool(name="psum", bufs=4, space=bass.MemorySpace.PSUM) as psum,
        ):
            # ---- AllGather the packed slice to every core (DRAM bounce) ----
            in_b = dram.tile([PACKP, S], BF16, tag="in_b")
            out_b = dram.tile([NCORES * PACKP, S], BF16, tag="out_b")
            nc.gpsimd.dma_start(in_b[:], pack_d[:])
            nc.gpsimd.collective_compute(
                "AllGather",
                ALU.bypass,
                replica_groups=[list(range(NCORES))],
                ins=[in_b.opt()],
                outs=[out_b.opt()],
            )

            # ---- candidate-side operands (full batch, from the gather) ----
            # Separate tiles per row group so every engine op starts at
            # partition 0 (mid-tile partition offsets fail BIR verification).
            full_e = sb.tile([D, B], BF16, tag="full_e")
            full_oh = sb.tile([NCLASS, B], BF16, tag="full_oh")
            full_sq = sb.tile([2, B], BF16, tag="full_sq")
            for c in range(NCORES):
                r = c * PACKP
                nc.sync.dma_start(full_e[:, c * S:(c + 1) * S],
                                  out_b[r:r + D, :])
                nc.sync.dma_start(full_oh[:, c * S:(c + 1) * S],
                                  out_b[r + D:r + D + NCLASS, :])
                nc.sync.dma_start(full_sq[:, c * S:(c + 1) * S],
                                  out_b[r + D + NCLASS:r + PACKP, :])

            # ---- anchor-side operands (this core's own slice) ----
            eloc = sb.tile([D, S], BF16, tag="eloc")
            nc.sync.dma_start(eloc[:], pack_d[0:D, :])
            xoh = sb.tile([NCLASS, S], BF16, tag="xoh")
            nc.sync.dma_start(xoh[:], pack_d[D:D + NCLASS, :])

            lhs_e = sb.tile([D, S], BF16, tag="lhs_e")
            nc.vector.tensor_scalar_mul(lhs_e[:], eloc[:], 2.0)
            lhs_oh = sb.tile([NCLASS, S], BF16, tag="lhs_oh")
            nc.vector.tensor_scalar_mul(lhs_oh[:], xoh[:], -BIG)
            ones2 = sb.tile([2, S], BF16, tag="ones2")
            nc.vector.memset(ones2[:], 1.0)

            # ---- main loop: 8 anchor tiles x 16 j-chunks ----
            hn_all = sb.tile([128, NAT * NJC], F32, tag="hn_all")
            hp_all = sb.tile([128, NAT * NJC], F32, tag="hp_all")
            for t in range(NAT):
                a0 = t * 128
                for k in range(NJC):
                    j0 = k * JC
                    ps = psum.tile([128, JC], F32, tag="ps")
                    nc.tensor.matmul(ps[:], lhs_e[:, a0:a0 + 128],
                                     full_e[:, j0:j0 + JC],
                                     start=True, stop=False)
                    nc.tensor.matmul(ps[:], lhs_oh[:, a0:a0 + 128],
                                     full_oh[:, j0:j0 + JC],
                                     start=False, stop=False)
                    nc.tensor.matmul(ps[:], ones2[:, a0:a0 + 128],
                                     full_sq[:, j0:j0 + JC],
                                     start=False, stop=True)
                    col = t * NJC + k
                    nc.vector.tensor_reduce(hn_all[:, col:col + 1], ps[:],
                                            axis=mybir.AxisListType.X,
                                            op=ALU.max)
                    nc.vector.tensor_reduce(hp_all[:, col:col + 1], ps[:],
                                            axis=mybir.AxisListType.X,
                                            op=ALU.min)

            # ---- fold chunks, ship [128, 16] ----
            res_sb = sb.tile([128, 2 * NAT], F32, tag="res_sb")
            for t in range(NAT):
                nc.vector.tensor_reduce(res_sb[:, t:t + 1],
                                        hn_all[:, t * NJC:(t + 1) * NJC],
                                        axis=mybir.AxisListType.X, op=ALU.max)
                nc.vector.tensor_reduce(res_sb[:, NAT + t:NAT + t + 1],
                                        hp_all[:, t * NJC:(t + 1) * NJC],
                                        axis=mybir.AxisListType.X, op=ALU.min)
            nc.sync.dma_start(res_d[:], res_sb[:])
    nc.compile()
    return nc


def prepare(embeddings, labels):
    emb = np.ascontiguousarray(np.asarray(embeddings, dtype=np.float32))
    lab = np.asarray(labels).ravel().astype(np.int64)
    assert emb.shape == (B, D) and lab.shape == (B,)
    sq = np.sum(emb.astype(np.float64) ** 2, axis=1)          # [B] f64
    oh = (lab[None, :] == np.arange(NCLASS)[:, None])         # [10, B]
    m1 = (-sq).astype(BFNP)                                   # bf16 hi limb
    m2 = (-sq - m1.astype(np.float64)).astype(BFNP)           # bf16 lo limb
    in_maps = []
    for i in range(NCORES):
        sl = slice(i * S, (i + 1) * S)
        pack = np.empty((PACKP, S), dtype=BFNP)
        pack[0:D] = emb[sl].T.astype(BFNP)
        pack[D:D + NCLASS] = oh[:, sl].astype(BFNP)
        pack[D + NCLASS] = m1[sl]
        pack[D + NCLASS + 1] = m2[sl]
        in_maps.append({"pack": pack})
    return in_maps, sq


def combine(results, sq):
    total = 0.0
    for i in range(NCORES):
        res = np.asarray(results[i]["res"], np.float32).astype(np.float64)
        # res[p, t] covers local anchor t*128+p -> transpose to local order
        hn_m = res[:, 0:NAT].T.reshape(S)
        hp_m = res[:, NAT:2 * NAT].T.reshape(S) + BIG
        sq_a = sq[i * S:(i + 1) * S]
        hn = np.sqrt(np.maximum(sq_a - hn_m, 0.0))
        hp = np.sqrt(np.maximum(sq_a - hp_m, 0.0))
        total += float(np.sum(np.maximum(hp - hn + MARGIN, 0.0)))
    return np.asarray(total / B, dtype=np.float32)


def kernel(embeddings, labels):
    global _NC_CACHE
    in_maps, sq = prepare(embeddings, labels)
    if _NC_CACHE is None:
        _NC_CACHE = build_nc()
    res = run_bass_kernel_spmd(_NC_CACHE, in_maps, list(range(NCORES)))
    return combine(res.results, sq)


# revision 7
# speedup vs baseline: 2.0628x; 2.0628x over previous
"""BatchHardTripletLoss on 8 TRN2 NeuronCores (Bass/Tile).

Wall-clock through the axon tunnel is transfer-dominated (~40 MB/s), so the
design minimizes bytes on the wire:

  - Each core ships ONE bf16 tensor pack[144, 1024] holding its 1/8 of the
    batch: rows 0..127 = e^T, rows 128..137 = one-hot(label), rows 138..139 =
    -||e||^2 split into two bf16 limbs, rows 140..143 = 0.  288 KB/core vs
    11.1 MB/core for a replicated layout.
  - On chip, an AllGather (DRAM bounce buffers) rebuilds the full candidate
    matrices [128, 8192] + [16, 8192] on every core.
  - One matmul pair per (anchor-tile, 512-wide j-chunk) computes
        val[a, j] = 2 e_a.e_j - sq_j - BIG * same(a, j)
    by augmenting the contraction: main matmul contracts the 128 embedding
    rows (lhs = 2*e_a), aux matmul contracts the 16 aux rows
    (lhs = [-BIG*onehot_a; 1; 1; 0*4], rhs = [onehot_j; -sq_hi; -sq_lo; 0*4])
    accumulating into the same PSUM bank.  Since masked same-class entries
    (~ -32.7k) always sit far below real different-class values (>= -600),
        reduce_max_j val  -> hardest-negative partial  max_j(2e.e - sq_j)
        reduce_min_j val  -> hardest-positive partial  min_j(2e.e - sq_j) - BIG
    i.e. ONE value matrix serves both reductions.
  - Output is just [128, 16] f32 per core (hn/hp partials per anchor tile);
    the host finishes with sqrt/relu/mean in float64:
        hn = sqrt(sq_a - hn_m),  hp = sqrt(sq_a - (hp_m + BIG)),
        loss = mean(relu(hp - hn + 1)).
"""

import numpy as np
import ml_dtypes

import jax

# The bass_exec path in neuronx_cc_hook has no NEFF cache: every fresh
# jax.jit closure inside run_bass_kernel_spmd re-runs compile_bir_kernel
# (~150ms+ per call).  JAX's persistent compilation cache short-circuits
# that: warm calls load the compiled executable from disk and never reach
# the hook.
jax.config.update("jax_compilation_cache_dir", "/tmp/jax_comp_cache")
jax.config.update("jax_persistent_cache_min_compile_time_secs", 0.0)
jax.config.update("jax_persistent_cache_min_entry_size_bytes", 0)

import concourse.bass as bass
import concourse.bacc as bacc
import concourse.tile as tile
from concourse import mybir
from concourse.bass_utils import run_bass_kernel_spmd

B, D, NCLASS = 8192, 128, 10
NCORES = 8
S = B // NCORES            # 1024 rows per core
NAUX = 12                  # aux rows: 10 one-hot + 2 sq limbs
PACKP = D + NAUX           # 140
NAT = S // 128             # 8 anchor tiles per core
JC = 512                   # j-chunk width (one PSUM bank)
NJC = B // JC              # 16 chunks
BIG = 32768.0              # exact in bf16 (2^15)
MARGIN = 1.0
F32 = mybir.dt.float32
BF16 = mybir.dt.bfloat16
ALU = mybir.AluOpType
BFNP = np.dtype(ml_dtypes.bfloat16)

_NC_CACHE = None


def build_nc():
    nc = bacc.Bacc(num_devices=NCORES)
    pack_d = nc.dram_tensor("pack", [PACKP, S], BF16, kind="ExternalInput")
    res_d = nc.dram_tensor("res", [128, 2 * NAT], F32, kind="ExternalOutput")

    with tile.TileContext(nc) as tc:
        with (
            tc.tile_pool(name="sb", bufs=1) as sb,
            tc.tile_pool(name="dram", bufs=1, space="DRAM") as dram,
            tc.tile_pool(name="psum", bufs=4, space=bass.MemorySpace.PSUM) as psum,
        ):
            # ---- AllGather the packed slice to every core (DRAM bounce) ----
            in_b = dram.tile([PACKP, S], BF16, tag="in_b")
            out_b = dram.tile([NCORES * PACKP, S], BF16, tag="out_b")
            nc.gpsimd.dma_start(in_b[:], pack_d[:])
            nc.gpsimd.collective_compute(
                "AllGather",
                ALU.bypass,
                replica_groups=[list(range(NCORES))],
                ins=[in_b.opt()],
                outs=[out_b.opt()],
            )

            # ---- candidate-side operands (full batch, from the gather) ----
            # Separate tiles per row group so every engine op starts at
            # partition 0 (mid-tile partition offsets fail BIR verification).
            full_e = sb.tile([D, B], BF16, tag="full_e")
            full_oh = sb.tile([NCLASS, B], BF16, tag="full_oh")
            full_sq = sb.tile([2, B], BF16, tag="full_sq")
            for c in range(NCORES):
                r = c * PACKP
                nc.sync.dma_start(full_e[:, c * S:(c + 1) * S],
                                  out_b[r:r + D, :])
                nc.sync.dma_start(full_oh[:, c * S:(c + 1) * S],
                                  out_b[r + D:r + D + NCLASS, :])
                nc.sync.dma_start(full_sq[:, c * S:(c + 1) * S],
                                  out_b[r + D + NCLASS:r + PACKP, :])

            # ---- anchor-side operands (this core's own slice) ----
            eloc = sb.tile([D, S], BF16, tag="eloc")
            nc.sync.dma_start(eloc[:], pack_d[0:D, :])
            xoh = sb.tile([NCLASS, S], BF16, tag="xoh")
            nc.sync.dma_start(xoh[:], pack_d[D:D + NCLASS, :])

            lhs_e = sb.tile([D, S], BF16, tag="lhs_e")
            nc.vector.tensor_scalar_mul(lhs_e[:], eloc[:], 2.0)
            lhs_oh = sb.tile([NCLASS, S], BF16, tag="lhs_oh")
            nc.vector.tensor_scalar_mul(lhs_oh[:], xoh[:], -BIG)
            ones2 = sb.tile([2, S], BF16, tag="ones2")
            nc.vector.memset(ones2[:], 1.0)

            # ---- main loop: 8 anchor tiles x 16 j-chunks ----
            hn_all = sb.tile([128, NAT * NJC], F32, tag="hn_all")
            hp_all = sb.tile([128, NAT * NJC], F32, tag="hp_all")
            for t in range(NAT):
                a0 = t * 128
                for k in range(NJC):
                    j0 = k * JC
                    ps = psum.tile([128, JC], F32, tag="ps")
                    nc.tensor.matmul(ps[:], lhs_e[:, a0:a0 + 128],
                                     full_e[:, j0:j0 + JC],
                                     start=True, stop=False)
                    nc.tensor.matmul(ps[:], lhs_oh[:, a0:a0 + 128],
                                     full_oh[:, j0:j0 + JC],
                                     start=False, stop=False)
                    nc.tensor.matmul(ps[:], ones2[:, a0:a0 + 128],
                                     full_sq[:, j0:j0 + JC],
                                     start=False, stop=True)
                    col = t * NJC + k
                    nc.vector.tensor_reduce(hn_all[:, col:col + 1], ps[:],
                                            axis=mybir.AxisListType.X,
                                            op=ALU.max)
                    nc.vector.tensor_reduce(hp_all[:, col:col + 1], ps[:],
                                            axis=mybir.AxisListType.X,
                                            op=ALU.min)

            # ---- fold chunks, ship [128, 16] ----
            res_sb = sb.tile([128, 2 * NAT], F32, tag="res_sb")
            for t in range(NAT):
                nc.vector.tensor_reduce(res_sb[:, t:t + 1],
                                        hn_all[:, t * NJC:(t + 1) * NJC],
                                        axis=mybir.AxisListType.X, op=ALU.max)
                nc.vector.tensor_reduce(res_sb[:, NAT + t:NAT + t + 1],
                                        hp_all[:, t * NJC:(t + 1) * NJC],
                                        axis=mybir.AxisListType.X, op=ALU.min)
            nc.sync.dma_start(res_d[:], res_sb[:])
    nc.compile()
    return nc


def prepare(embeddings, labels):
    emb = np.ascontiguousarray(np.asarray(embeddings, dtype=np.float32))
    lab = np.asarray(labels).ravel().astype(np.int64)
    assert emb.shape == (B, D) and lab.shape == (B,)
    sq = np.sum(emb.astype(np.float64) ** 2, axis=1)          # [B] f64
    oh = (lab[None, :] == np.arange(NCLASS)[:, None])         # [10, B]
    m1 = (-sq).astype(BFNP)                                   # bf16 hi limb
    m2 = (-sq - m1.astype(np.float64)).astype(BFNP)           # bf16 lo limb
    in_maps = []
    for i in range(NCORES):
        sl = slice(i * S, (i + 1) * S)
        pack = np.empty((PACKP, S), dtype=BFNP)
        pack[0:D] = emb[sl].T.astype(BFNP)
        pack[D:D + NCLASS] = oh[:, sl].astype(BFNP)
        pack[D + NCLASS] = m1[sl]
        pack[D + NCLASS + 1] = m2[sl]
        in_maps.append({"pack": pack})
    return in_maps, sq


def combine(results, sq):
    total = 0.0
    for i in range(NCORES):
        res = np.asarray(results[i]["res"], np.float32).astype(np.float64)
        # res[p, t] covers local anchor t*128+p -> transpose to local order
        hn_m = res[:, 0:NAT].T.reshape(S)
        hp_m = res[:, NAT:2 * NAT].T.reshape(S) + BIG
        sq_a = sq[i * S:(i + 1) * S]
        hn = np.sqrt(np.maximum(sq_a - hn_m, 0.0))
        hp = np.sqrt(np.maximum(sq_a - hp_m, 0.0))
        total += float(np.sum(np.maximum(hp - hn + MARGIN, 0.0)))
    return np.asarray(total / B, dtype=np.float32)


def kernel(embeddings, labels):
    global _NC_CACHE
    in_maps, sq = prepare(embeddings, labels)
    if _NC_CACHE is None:
        _NC_CACHE = build_nc()
    res = run_bass_kernel_spmd(_NC_CACHE, in_maps, list(range(NCORES)))
    return combine(res.results, sq)


# revision 15
# speedup vs baseline: 2.7172x; 1.3172x over previous
"""BatchHardTripletLoss on 8 TRN2 NeuronCores (Bass/Tile).

Wall-clock through the axon tunnel is transfer-dominated (~40 MB/s), so the
design minimizes bytes on the wire:

  - Each core ships ONE bf16 tensor pack[144, 1024] holding its 1/8 of the
    batch: rows 0..127 = e^T, rows 128..137 = one-hot(label), rows 138..139 =
    -||e||^2 split into two bf16 limbs, rows 140..143 = 0.  288 KB/core vs
    11.1 MB/core for a replicated layout.
  - On chip, an AllGather (DRAM bounce buffers) rebuilds the full candidate
    matrices [128, 8192] + [16, 8192] on every core.
  - One matmul pair per (anchor-tile, 512-wide j-chunk) computes
        val[a, j] = 2 e_a.e_j - sq_j - BIG * same(a, j)
    by augmenting the contraction: main matmul contracts the 128 embedding
    rows (lhs = 2*e_a), aux matmul contracts the 16 aux rows
    (lhs = [-BIG*onehot_a; 1; 1; 0*4], rhs = [onehot_j; -sq_hi; -sq_lo; 0*4])
    accumulating into the same PSUM bank.  Since masked same-class entries
    (~ -32.7k) always sit far below real different-class values (>= -600),
        reduce_max_j val  -> hardest-negative partial  max_j(2e.e - sq_j)
        reduce_min_j val  -> hardest-positive partial  min_j(2e.e - sq_j) - BIG
    i.e. ONE value matrix serves both reductions.
  - Output is just [128, 16] f32 per core (hn/hp partials per anchor tile);
    the host finishes with sqrt/relu/mean in float64:
        hn = sqrt(sq_a - hn_m),  hp = sqrt(sq_a - (hp_m + BIG)),
        loss = mean(relu(hp - hn + 1)).
"""

import numpy as np
import ml_dtypes

import jax

# The bass_exec path in neuronx_cc_hook has no NEFF cache: every fresh
# jax.jit closure inside run_bass_kernel_spmd re-runs compile_bir_kernel
# (~150ms+ per call).  JAX's persistent compilation cache short-circuits
# that: warm calls load the compiled executable from disk and never reach
# the hook.
jax.config.update("jax_compilation_cache_dir", "/tmp/jax_comp_cache")
jax.config.update("jax_persistent_cache_min_compile_time_secs", 0.0)
jax.config.update("jax_persistent_cache_min_entry_size_bytes", 0)

import concourse.bass as bass
import concourse.bacc as bacc
import concourse.tile as tile
from concourse import mybir
from concourse.bass_utils import run_bass_kernel_spmd

B, D, NCLASS = 8192, 128, 10
NCORES = 8
S = B // NCORES            # 1024 rows per core
NAUX = 13                  # aux rows: 10 one-hot + 3 sq limbs
PACKP = D + NAUX           # 141
NAT = S // 128             # 8 anchor tiles per core
JC = 512                   # j-chunk width (one PSUM bank)
NJC = B // JC              # 16 chunks
BIG = 32768.0              # exact in bf16 (2^15)
MARGIN = 1.0
F32 = mybir.dt.float32
BF16 = mybir.dt.bfloat16
FP8 = mybir.dt.float8e4
ALU = mybir.AluOpType
BFNP = np.dtype(ml_dtypes.bfloat16)
F8NP = np.dtype(ml_dtypes.float8_e4m3)

_NC_CACHE = None


def build_nc():
    nc = bacc.Bacc(num_devices=NCORES)
    pack_d = nc.dram_tensor("pack", [PACKP, S], FP8, kind="ExternalInput")
    res_d = nc.dram_tensor("res", [128, 2 * NAT], F32, kind="ExternalOutput")

    with tile.TileContext(nc) as tc:
        with (
            tc.tile_pool(name="sb", bufs=1) as sb,
            tc.tile_pool(name="dram", bufs=1, space="DRAM") as dram,
            tc.tile_pool(name="psum", bufs=4, space=bass.MemorySpace.PSUM) as psum,
        ):
            # ---- AllGather the packed slice to every core (DRAM bounce) ----
            in_b = dram.tile([PACKP, S], FP8, tag="in_b")
            out_b = dram.tile([NCORES * PACKP, S], FP8, tag="out_b")
            nc.gpsimd.dma_start(in_b[:], pack_d[:])
            nc.gpsimd.collective_compute(
                "AllGather",
                ALU.bypass,
                replica_groups=[list(range(NCORES))],
                ins=[in_b.opt()],
                outs=[out_b.opt()],
            )

            # ---- candidate-side operands (full batch, from the gather) ----
            # Separate tiles per row group so every engine op starts at
            # partition 0 (mid-tile partition offsets fail BIR verification).
            # e stays fp8 (matmul operand only); one-hot and sq rows are
            # widened to bf16 on chip so the mask scale (-BIG/2) fits.
            full_e = sb.tile([D, B], FP8, tag="full_e")
            full_oh = sb.tile([NCLASS, B], FP8, tag="full_oh")
            full_sq = sb.tile([3, B], FP8, tag="full_sq")
            for c in range(NCORES):
                r = c * PACKP
                nc.sync.dma_start(full_e[:, c * S:(c + 1) * S],
                                  out_b[r:r + D, :])
                nc.sync.dma_start(full_oh[:, c * S:(c + 1) * S],
                                  out_b[r + D:r + D + NCLASS, :])
                nc.sync.dma_start(full_sq[:, c * S:(c + 1) * S],
                                  out_b[r + D + NCLASS:r + PACKP, :])
            roh = sb.tile([NCLASS, B], BF16, tag="roh")
            nc.vector.tensor_copy(roh[:], full_oh[:])
            rsq = sb.tile([3, B], BF16, tag="rsq")
            nc.vector.tensor_copy(rsq[:], full_sq[:])

            # ---- anchor-side operands (this core's own slice) ----
            # lhs for the main matmul is the raw fp8 e slice: the kernel
            # computes val2 = e.e - sq/2 - (BIG/2)*same and the host doubles.
            eloc = sb.tile([D, S], FP8, tag="eloc")
            nc.sync.dma_start(eloc[:], pack_d[0:D, :])
            xoh = sb.tile([NCLASS, S], FP8, tag="xoh")
            nc.sync.dma_start(xoh[:], pack_d[D:D + NCLASS, :])

            lhs_oh = sb.tile([NCLASS, S], BF16, tag="lhs_oh")
            nc.vector.tensor_scalar_mul(lhs_oh[:], xoh[:], -BIG / 2)
            ones3 = sb.tile([3, S], BF16, tag="ones3")
            nc.vector.memset(ones3[:], 1.0)

            # ---- main loop: 8 anchor tiles x 16 j-chunks ----
            hn_all = sb.tile([128, NAT * NJC], F32, tag="hn_all")
            hp_all = sb.tile([128, NAT * NJC], F32, tag="hp_all")
            for t in range(NAT):
                a0 = t * 128
                for k in range(NJC):
                    j0 = k * JC
                    ps = psum.tile([128, JC], F32, tag="ps")
                    nc.tensor.matmul(ps[:], eloc[:, a0:a0 + 128],
                                     full_e[:, j0:j0 + JC],
                                     start=True, stop=False)
                    nc.tensor.matmul(ps[:], lhs_oh[:, a0:a0 + 128],
                                     roh[:, j0:j0 + JC],
                                     start=False, stop=False)
                    nc.tensor.matmul(ps[:], ones3[:, a0:a0 + 128],
                                     rsq[:, j0:j0 + JC],
                                     start=False, stop=True)
                    col = t * NJC + k
                    nc.vector.tensor_reduce(hn_all[:, col:col + 1], ps[:],
                                            axis=mybir.AxisListType.X,
                                            op=ALU.max)
                    nc.vector.tensor_reduce(hp_all[:, col:col + 1], ps[:],
                                            axis=mybir.AxisListType.X,
                                            op=ALU.min)

            # ---- fold chunks, ship [128, 16] ----
            res_sb = sb.tile([128, 2 * NAT], F32, tag="res_sb")
            for t in range(NAT):
                nc.vector.tensor_reduce(res_sb[:, t:t + 1],
                                        hn_all[:, t * NJC:(t + 1) * NJC],
                                        axis=mybir.AxisListType.X, op=ALU.max)
                nc.vector.tensor_reduce(res_sb[:, NAT + t:NAT + t + 1],
                                        hp_all[:, t * NJC:(t + 1) * NJC],
                                        axis=mybir.AxisListType.X, op=ALU.min)
            nc.sync.dma_start(res_d[:], res_sb[:])
    nc.compile()
    return nc


def prepare(embeddings, labels):
    emb = np.ascontiguousarray(np.asarray(embeddings, dtype=np.float32))
    lab = np.asarray(labels).ravel().astype(np.int64)
    assert emb.shape == (B, D) and lab.shape == (B,)
    sq = np.sum(emb.astype(np.float64) ** 2, axis=1)          # [B] f64
    oh = (lab[None, :] == np.arange(NCLASS)[:, None])         # [10, B]
    h = -sq / 2                                               # 3 fp8 limbs
    m1 = h.astype(F8NP)
    r = h - m1.astype(np.float64)
    m2 = r.astype(F8NP)
    m3 = (r - m2.astype(np.float64)).astype(F8NP)
    in_maps = []
    for i in range(NCORES):
        sl = slice(i * S, (i + 1) * S)
        pack = np.empty((PACKP, S), dtype=F8NP)
        pack[0:D] = emb[sl].T.astype(F8NP)
        pack[D:D + NCLASS] = oh[:, sl].astype(F8NP)
        pack[D + NCLASS] = m1[sl]
        pack[D + NCLASS + 1] = m2[sl]
        pack[D + NCLASS + 2] = m3[sl]
        in_maps.append({"pack": pack})
    return in_maps, sq


def combine(results, sq):
    total = 0.0
    for i in range(NCORES):
        res = np.asarray(results[i]["res"], np.float32).astype(np.float64)
        # res[p, t] covers local anchor t*128+p -> transpose to local order.
        # The kernel computed val2 = (val)/2, so double, and undo -BIG*same.
        hn_m = 2.0 * res[:, 0:NAT].T.reshape(S)
        hp_m = 2.0 * res[:, NAT:2 * NAT].T.reshape(S) + BIG
        sq_a = sq[i * S:(i + 1) * S]
        hn = np.sqrt(np.maximum(sq_a - hn_m, 0.0))
        hp = np.sqrt(np.maximum(sq_a - hp_m, 0.0))
        total += float(np.sum(np.maximum(hp - hn + MARGIN, 0.0)))
    return np.asarray(total / B, dtype=np.float32)


def kernel(embeddings, labels):
    global _NC_CACHE
    in_maps, sq = prepare(embeddings, labels)
    if _NC_CACHE is None:
        _NC_CACHE = build_nc()
    res = run_bass_kernel_spmd(_NC_CACHE, in_maps, list(range(NCORES)))
    return combine(res.results, sq)
